# revision 24
# baseline (speedup 1.0000x reference)
"""Multi-head self-attention (B=2, T=2048, D=1024, H=16) on 8 TRN2 NeuronCores.

Sharding: batch x head-group (Megatron column split of W_qkv, row split of
W_proj). Core c handles batch b = c//4 and heads h0 = 4*(c%4) .. h0+4; the
host sums the 4 partial output projections per batch and adds b_proj.

v2 (vs the fp32r baseline at 214us):
  - bf16 everywhere on the PE: x/W_qkv/W_v/W_proj are cast + pre-tiled on
    the host into [128, ...] partition-major layouts (8KB contiguous
    per-partition DMA lines instead of 2KB, ~half the input bytes). bf16
    also enables Fast Weight Load (fp32 LDWEIGHTS was rate-limiting the
    N=256 v-projection matmuls) and 1 cyc/row at any N, so the causal
    diagonal tiles no longer need the N>=256 widening that fp32r required.
  - globally interleaved emission: the four qkv t-quarters, the causal
    attention blocks (qi), and the output-projection blocks are emitted as
    one pipeline. Attention j-tiles produce an ACT(exp)-vs-PE deficit and
    ready PE work (next quarter's matmuls, previous block's projection) is
    injected between j-tiles to cover it, so the PE never idles waiting on
    softmax exps. This keeps PE duty ~100%: the fp32r baseline oscillated
    HAM (4/8 <-> 8/8 clock) through all of phase 2 because PE duty was
    ~60%, and it serialized startup-DMA / qkv / attention / projection.
  - engine placement: exp is ScalarE-only (the true phase-2 roofline,
    ~65us); masks/evacuations/normalize on VectorE; input x + all output
    DMAs on the GpSimd queue; weights + rec lane-shifts on the Sync queue.

Per-core PE streaming floor ~100us (qkv 41 + S^T 15 (64-row-tiled pairs)
+ ctx 29 + proj 14); ScalarE exp ~65us hides under it when interleaved.
"""

import sys

if "/opt/trn_rl_repo" not in sys.path:
    sys.path.insert(0, "/opt/trn_rl_repo")

from collections import deque
from contextlib import ExitStack

import numpy as np
import ml_dtypes

import concourse.bass as bass
import concourse.bacc as bacc
import concourse.tile as tile
from concourse import mybir
from concourse.bass_utils import run_bass_kernel_spmd

B, T, D, H, DH = 2, 2048, 1024, 16, 64
NCORES = 8
HL = 4          # heads per core
P = 128         # SBUF/PSUM partitions
QT = 512        # q tile (PSUM bank, moving free dim)
KT = 128        # kv tile (PSUM partition dim)
F32 = mybir.dt.float32
BF16 = mybir.dt.bfloat16
BF16NP = ml_dtypes.bfloat16

# emission-time engine-cost model (ns) for the deficit scheduler
def _act_cost(w):        # one 2-head exp over w columns (measured ovh ~205cyc)
    return (2 * w + 205) / 1.2

def _pe_tile_cost(w):    # S^T pair (row-tiled, concurrent) + 2 ctx matmuls
    return (w + 2 * w) / 2.4 + 80


def _build_nc() -> bass.Bass:
    nc = bacc.Bacc(None)
    Exp = mybir.ActivationFunctionType.Exp

    # host-pre-tiled inputs (partition-major, large contiguous lines)
    xt_d = nc.dram_tensor("xt", [P, 4, 8, QT], BF16, kind="ExternalInput")
    wqk_d = nc.dram_tensor("wqk", [P, 8, 512], BF16, kind="ExternalInput")
    wv_d = nc.dram_tensor("wv", [P, 8, 256], BF16, kind="ExternalInput")
    wp_d = nc.dram_tensor("wp", [P, 2, D], BF16, kind="ExternalInput")
    # bqk padded to 1KB rows: 16B-row descriptors stalled the DMA queue 8us
    bqk_d = nc.dram_tensor("bqk", [P, 256], F32, kind="ExternalInput")
    bv_d = nc.dram_tensor("bv", [256], F32, kind="ExternalInput")
    out_d = nc.dram_tensor("out", [T, D], BF16, kind="ExternalOutput")

    kv = np.arange(KT)
    # 0/1 keep-mask (bf16): applied multiplicatively to P AFTER the exp, so
    # the ScalarE exp never waits on a VectorE mask add.
    tril01_np = (kv[:, None] <= kv[None, :]).astype(BF16NP)
    tril_d = nc.inline_tensor(tril01_np, name="tril01")

    with tile.TileContext(nc) as tc, ExitStack() as ctx:
        perm = ctx.enter_context(tc.tile_pool(name="perm", bufs=1))
        ppool = ctx.enter_context(tc.tile_pool(name="ppool", bufs=6))
        rpool = ctx.enter_context(tc.tile_pool(name="rpool", bufs=3))
        opool = ctx.enter_context(tc.tile_pool(name="opool", bufs=3))
        # separate PSUM rings: S tiles (4KB x2) must not share slots with
        # qkv/proj accumulators (2KB x2) — a shared ring makes every filler
        # allocation WAR-wait on an exp two slots back, collapsing the
        # attention pipeline depth. psC: ctx accumulators (2KB x2). 16KB total.
        psS = ctx.enter_context(tc.tile_pool(name="psS", bufs=2, space="PSUM"))
        psA = ctx.enter_context(tc.tile_pool(name="psA", bufs=2, space="PSUM"))
        psC = ctx.enter_context(tc.tile_pool(name="psC", bufs=2, space="PSUM"))

        # --- startup DMAs (flattened to one free dim for 4-8KB descriptors;
        # MM0 needs wqk half A + xq0: each split to its own queue so the
        # first matmul starts ~13us in (7us of that is framework preamble))
        wqk_sb = perm.tile([P, 8, 512], BF16)
        nc.sync.dma_start(
            out=wqk_sb[:, 0:4].rearrange("p a j -> p (a j)"),
            in_=wqk_d[:, 0:4].rearrange("p a j -> p (a j)"),
        )
        xq_sb = []
        for tt4 in range(4):
            xq = perm.tile([P, 8, QT], BF16, name=f"xq{tt4}")
            xq_sb.append(xq)
        nc.scalar.dma_start(
            out=xq_sb[0][:, 0:4].rearrange("p a t -> p (a t)"),
            in_=xt_d[:, 0, 0:4].rearrange("p a t -> p (a t)"),
        )
        nc.gpsimd.dma_start(
            out=xq_sb[0][:, 4:8].rearrange("p a t -> p (a t)"),
            in_=xt_d[:, 0, 4:8].rearrange("p a t -> p (a t)"),
        )
        nc.scalar.dma_start(
            out=wqk_sb[:, 4:8].rearrange("p a j -> p (a j)"),
            in_=wqk_d[:, 4:8].rearrange("p a j -> p (a j)"),
        )
        for tt4 in range(1, 4):
            nc.gpsimd.dma_start(
                out=xq_sb[tt4].rearrange("p a t -> p (a t)"),
                in_=xt_d[:, tt4].rearrange("p a t -> p (a t)"),
            )
        wv_sb = perm.tile([P, 8, 256], BF16)
        nc.sync.dma_start(
            out=wv_sb.rearrange("p a j -> p (a j)"),
            in_=wv_d[...].rearrange("p a j -> p (a j)"),
        )
        bqk_sb = perm.tile([P, 256], F32)
        nc.sync.dma_start(out=bqk_sb, in_=bqk_d[...])
        bv_sb = perm.tile([P, 256], F32)
        nc.sync.dma_start(
            out=bv_sb, in_=bass.AP(tensor=bv_d, offset=0, ap=[[0, P], [1, 256]])
        )
        tril_sb = perm.tile([P, KT], BF16)
        nc.sync.dma_start(out=tril_sb, in_=tril_d[...])
        wp_sb = perm.tile([P, 2, D], BF16)
        nc.sync.dma_start(
            out=wp_sb.rearrange("p a e -> p (a e)"),
            in_=wp_d[...].rearrange("p a e -> p (a e)"),
        )

        # first Exp triggers a ~2.7us ACT table load; fire it early
        expwarm = perm.tile([P, 8], F32)
        nc.scalar.activation(out=expwarm, in_=tril_sb[:, 0:8], func=Exp)

        qk_sb = perm.tile([P, 4, T], BF16)           # jt 0,1: Q^T; 2,3: K^T
        vaug_sb = perm.tile([P, 16, HL, 128], BF16)  # [kv, tt, h, V|ones]
        vaug_hh = vaug_sb.rearrange("p t (hp hh) c -> p t hp hh c", hh=2)
        ctxn_sb = perm.tile([P, 2, T], BF16)         # normalized ctx^T

        # ones halves of vaug never change: write them once.
        # even heads are [V|ones], odd heads [ones|V] (parity keeps ctx rows
        # lane-aligned with the ctxn head-pair packing; see normalize()).
        for par, sl in ((0, slice(64, 128)), (1, slice(0, 64))):
            src = bass.AP(
                tensor=tril_sb.tensor, offset=tril_sb.offset,
                ap=[tril_sb.ap[0], [0, 16], [0, 2], [1, 64]],
            )
            nc.vector.tensor_scalar(
                out=vaug_hh[:, :, :, par, sl], in0=src,
                scalar1=0.0, scalar2=1.0,
                op0=mybir.AluOpType.mult, op1=mybir.AluOpType.add,
            )

        # --- work units -------------------------------------------------
        def qk_jt(tt4, jt):
            """qkv projection, one 128-wide j chunk x 512 t: 8 MMs + evac."""
            ps = psA.tile([P, QT], F32, name=f"qkps{tt4}_{jt}", tag="acc")
            for dt in range(8):
                nc.tensor.matmul(
                    out=ps,
                    lhsT=wqk_sb[:, dt, jt * 128:(jt + 1) * 128],
                    rhs=xq_sb[tt4][:, dt, :],
                    start=(dt == 0),
                    stop=(dt == 7),
                )
            # PSUM->SBUF evac + bias on DVE: keeping this OFF ScalarE matters
            # (an Ident here head-of-line blocks ready exps in the ACT FIFO
            # behind this tile's still-accumulating matmuls)
            b = bqk_sb[:, jt:jt + 1]
            bias = bass.AP(tensor=b.tensor, offset=b.offset, ap=[b.ap[0], [0, QT]])
            nc.vector.tensor_add(
                out=qk_sb[:, jt, tt4 * QT:(tt4 + 1) * QT], in0=ps, in1=bias
            )

        def v_k(tt4, k):
            """v projection for one 128-t block: 8 MMs + vaug fills."""
            tt = tt4 * 4 + k
            psv = psA.tile([P, 256], F32, name=f"vps{tt}", tag="acc")
            for dt in range(8):
                nc.tensor.matmul(
                    out=psv,
                    lhsT=xq_sb[tt4][:, dt, k * 128:(k + 1) * 128],
                    rhs=wv_sb[:, dt, :],
                    start=(dt == 0),
                    stop=(dt == 7),
                )
            vview = vaug_hh[:, tt]
            pview = psv.rearrange("p (hp hh d) -> p hp hh d", hp=2, hh=2)
            bview = bv_sb.rearrange("p (hp hh d) -> p hp hh d", hp=2, hh=2)
            nc.vector.tensor_add(
                out=vview[:, :, 0, 0:64], in0=pview[:, :, 0, :], in1=bview[:, :, 0, :]
            )
            nc.vector.tensor_add(
                out=vview[:, :, 1, 64:128], in0=pview[:, :, 1, :], in1=bview[:, :, 1, :]
            )

        def proj_block(tt, fine=False):
            """output projection for one 128-t block + DMA out. fine=True
            (tail blocks) pipelines per-half evac+DMA to shorten the drain."""
            ob = opool.tile([P, D], BF16, name=f"ob{tt}", tag="ob")
            for et in range(2):
                ps = psA.tile([P, QT], F32, name=f"ops{tt}_{et}", tag="acc")
                for ft in range(2):
                    nc.tensor.matmul(
                        out=ps,
                        lhsT=ctxn_sb[:, ft, tt * KT:(tt + 1) * KT],
                        rhs=wp_sb[:, ft, et * QT:(et + 1) * QT],
                        start=(ft == 0),
                        stop=(ft == 1),
                    )
                nc.vector.tensor_copy(out=ob[:, et * QT:(et + 1) * QT], in_=ps)
                if fine:
                    q = nc.gpsimd if et == 0 else nc.scalar
                    q.dma_start(
                        out=out_d[tt * KT:(tt + 1) * KT, et * QT:(et + 1) * QT],
                        in_=ob[:, et * QT:(et + 1) * QT],
                    )
            if not fine:
                nc.gpsimd.dma_start(out=out_d[tt * KT:(tt + 1) * KT, :], in_=ob)

        # --- attention stages -------------------------------------------
        Th_by_blk = {}

        def stage_a(hp, qi, j):
            """S^T pair (64-row-tiled, concurrent) + mask + 2-head exp."""
            q0 = qi * QT
            qoff = max(0, KT * j - q0)
            w = QT - qoff
            s = psS.tile([P, 2, QT], F32, name=f"s{hp}_{qi}_{j}", tag="s")
            for hh in range(2):
                nc.tensor.matmul(
                    out=s[:, hh, qoff:QT],
                    lhsT=qk_sb[hh * 64:(hh + 1) * 64, 2 + hp, j * KT:(j + 1) * KT],
                    rhs=qk_sb[hh * 64:(hh + 1) * 64, hp, q0 + qoff:q0 + QT],
                    start=True,
                    stop=True,
                )
            p_t = ppool.tile([P, 2, QT], BF16, name=f"p{hp}_{qi}_{j}", tag="p")
            nc.scalar.activation(
                out=p_t[:, :, qoff:QT], in_=s[:, :, qoff:QT], func=Exp
            )
            if j >= 4 * qi:
                # diagonal tile: zero the above-diagonal P entries with a
                # bf16 0/1 multiply AFTER the exp (exp of those live scores
                # is finite), so ACT never waits on the DVE mask
                pm = p_t[:, :, qoff:qoff + KT]
                mask_b = bass.AP(
                    tensor=tril_sb.tensor, offset=tril_sb.offset,
                    ap=[tril_sb.ap[0], [0, 2], [1, KT]],
                )
                nc.vector.tensor_mul(out=pm, in0=pm, in1=mask_b)
            return p_t

        def stage_b(hp, qi, j, p_t):
            """[ctx|den] matmuls for both heads; normalize at last j."""
            q0 = qi * QT
            njt = 4 * qi + 4
            qoff = max(0, KT * j - q0)
            if j == 0:
                Th_by_blk[(hp, qi)] = [
                    psC.tile([P, QT], F32, name=f"T{hp}_{qi}_{hh}", tag="C")
                    for hh in range(2)
                ]
            Th = Th_by_blk[(hp, qi)]
            for hh in range(2):
                nc.tensor.matmul(
                    out=Th[hh][:, qoff:QT],
                    lhsT=vaug_sb[:, j, hp * 2 + hh, :],
                    rhs=p_t[:, hh, qoff:QT],
                    start=(j == 0),
                    stop=(j == njt - 1),
                )
            if j == njt - 1:
                normalize(hp, qi)

        def normalize(hp, qi):
            q0 = qi * QT
            Th = Th_by_blk.pop((hp, qi))
            for hh in range(2):
                cl = hh * 64          # ctx lanes base
                rec = rpool.tile([P, QT], F32, name=f"rec{hp}_{qi}_{hh}", tag="rec")
                # reciprocal_approx_fast mis-executes at partition base 64
                # (HW-verified), so always run it at base 0.
                if hh == 1:
                    nc.vector.reciprocal_approx_fast(out=rec[0:64, :], in_=Th[hh][0:64, :])
                    nc.sync.dma_start(out=rec[64:128, :], in_=rec[0:64, :])
                else:
                    nc.vector.tensor_copy(out=rec[64:128, :], in_=Th[hh][64:128, :])
                    nc.sync.dma_start(out=rec[0:64, :], in_=rec[64:128, :])
                    nc.vector.reciprocal_approx_fast(out=rec[0:64, :], in_=rec[0:64, :])
                nc.vector.tensor_mul(
                    out=ctxn_sb[cl:cl + 64, hp, q0:q0 + QT],
                    in0=Th[hh][cl:cl + 64, :],
                    in1=rec[cl:cl + 64, :],
                )

        # --- globally interleaved emission ------------------------------
        # fillers: (tag, pe_cost_ns, emit_fn) of ready PE work injected
        # between attention j-tiles to cover the ACT(exp) deficit. proj
        # blocks are held in a reserve and only spent inside attn(3) (the
        # qkv quarters cover attn(0..2)); attn(3) has no quarter left.
        fillers = deque()
        reserve = deque()

        def add_quarter_fillers(tt4):
            for jt in range(4):
                fillers.append((("q", tt4), 1750.0, lambda j=jt: qk_jt(tt4, j)))
            for k in range(4):
                fillers.append((("q", tt4), 950.0, lambda kk=k: v_k(tt4, kk)))

        def add_proj_reserve(qi):
            for tt in range(qi * 4, qi * 4 + 4):
                reserve.append(
                    (("pj", qi), 950.0, lambda t=tt: proj_block(t, fine=(qi == 0)))
                )

        def drain_tag(tag):
            while fillers and any(f[0] == tag for f in fillers):
                t, c, fn = fillers.popleft()
                fn()

        def attn_block(qi, use_reserve):
            # DEPTH=2 matches the 2-slot S ring; fillers go between stage_a
            # and stage_b so the pending exp gets PE-time cover
            DEPTH = 2
            lag = 0.0
            for hp in range(2):
                pend = deque()
                for j in range(4 * qi + 4):
                    p_t = stage_a(hp, qi, j)
                    pend.append((hp, qi, j, p_t))
                    w = QT - max(0, KT * j - qi * QT)
                    lag += _act_cost(w) - _pe_tile_cost(w)
                    while lag > 400.0 and (fillers or (use_reserve and reserve)):
                        t, c, fn = (fillers or reserve).popleft()
                        fn()
                        lag -= c
                    if len(pend) >= DEPTH:
                        stage_b(*pend.popleft())
                while pend:
                    stage_b(*pend.popleft())

        # Attention processed in order 1,2,3,0: the tiny qi=0 block runs
        # LAST (fed by proj(3) as filler) so the drain chain ends on the
        # smallest possible exp tail, and proj(1)/proj(2) are reserved for
        # exp-heavy attn(3). Quarter qi must be fully emitted before attn(qi).
        add_quarter_fillers(0)
        drain_tag(("q", 0))
        add_quarter_fillers(1)
        drain_tag(("q", 1))
        add_quarter_fillers(2)
        attn_block(1, use_reserve=False)
        drain_tag(("q", 2))
        add_quarter_fillers(3)
        attn_block(2, use_reserve=False)
        drain_tag(("q", 3))
        add_proj_reserve(1)
        add_proj_reserve(2)
        attn_block(3, use_reserve=True)
        add_proj_reserve(3)
        attn_block(0, use_reserve=True)
        add_proj_reserve(0)
        for t, c, fn in list(fillers) + list(reserve):
            fn()

    nc.finalize()
    return nc


_NC_CACHE: list = []


def _get_nc() -> bass.Bass:
    if not _NC_CACHE:
        _NC_CACHE.append(_build_nc())
    return _NC_CACHE[0]


def _shard_inputs(x, W_qkv, b_qkv, W_proj):
    scale = np.float32(1.0 / np.sqrt(DH))
    in_maps = []
    # x pre-tiled per batch: [p, quarter, dt, t] with 8KB contiguous lines
    xts = []
    for b in range(B):
        xt = x[b].T.reshape(8, P, 4, QT).transpose(1, 2, 0, 3)
        xts.append(np.ascontiguousarray(xt).astype(BF16NP))
    for c in range(NCORES):
        b = c // 4
        h0 = (c % 4) * HL
        lo = h0 * DH
        wqk = np.concatenate(
            [W_qkv[:, lo:lo + 256] * scale, W_qkv[:, D + lo:D + lo + 256]], axis=1
        )
        bqk = np.concatenate([b_qkv[lo:lo + 256] * scale, b_qkv[D + lo:D + lo + 256]])
        wv = W_qkv[:, 2 * D + lo:2 * D + lo + 256]
        wp = W_proj[lo:lo + 256, :]
        in_maps.append({
            "xt": xts[b],
            "wqk": np.ascontiguousarray(
                wqk.reshape(8, P, 512).transpose(1, 0, 2)).astype(BF16NP),
            "wv": np.ascontiguousarray(
                wv.reshape(8, P, 256).transpose(1, 0, 2)).astype(BF16NP),
            "wp": np.ascontiguousarray(
                wp.reshape(2, P, D).transpose(1, 0, 2)).astype(BF16NP),
            "bqk": np.ascontiguousarray(
                np.pad(bqk.reshape(4, P).T, ((0, 0), (0, 252))), np.float32),
            "bv": np.ascontiguousarray(b_qkv[2 * D + lo:2 * D + lo + 256], np.float32),
        })
    return in_maps


def kernel(x, W_qkv, b_qkv, W_proj, b_proj, _trace=False, _tmpdir=None):
    x = np.asarray(x, np.float32)
    W_qkv = np.asarray(W_qkv, np.float32)
    b_qkv = np.asarray(b_qkv, np.float32)
    W_proj = np.asarray(W_proj, np.float32)
    b_proj = np.asarray(b_proj, np.float32)

    nc = _get_nc()
    in_maps = _shard_inputs(x, W_qkv, b_qkv, W_proj)
    kw = {}
    if _trace:
        kw = dict(trace=True, tmpdir=_tmpdir)
    r = run_bass_kernel_spmd(nc, in_maps, core_ids=list(range(NCORES)), **kw)
    out = np.zeros((B, T, D), np.float32)
    for c in range(NCORES):
        out[c // 4] += np.asarray(r.results[c]["out"], np.float32)
    out += b_proj[None, None, :]
    if _trace:
        return out, r
    return out


# revision 32
# speedup vs baseline: 1.0464x; 1.0464x over previous
"""Multi-head self-attention (B=2, T=2048, D=1024, H=16) on 8 TRN2 NeuronCores.

Sharding: batch x head-group (Megatron column split of W_qkv, row split of
W_proj). Core c handles batch b = c//4 and heads h0 = 4*(c%4) .. h0+4; the
host sums the 4 partial output projections per batch and adds b_proj.

v2 (vs the fp32r baseline at 214us):
  - bf16 everywhere on the PE: x/W_qkv/W_v/W_proj are cast + pre-tiled on
    the host into [128, ...] partition-major layouts (8KB contiguous
    per-partition DMA lines instead of 2KB, ~half the input bytes). bf16
    also enables Fast Weight Load (fp32 LDWEIGHTS was rate-limiting the
    N=256 v-projection matmuls) and 1 cyc/row at any N, so the causal
    diagonal tiles no longer need the N>=256 widening that fp32r required.
  - globally interleaved emission: the four qkv t-quarters, the causal
    attention blocks (qi), and the output-projection blocks are emitted as
    one pipeline. Attention j-tiles produce an ACT(exp)-vs-PE deficit and
    ready PE work (next quarter's matmuls, previous block's projection) is
    injected between j-tiles to cover it, so the PE never idles waiting on
    softmax exps. This keeps PE duty ~100%: the fp32r baseline oscillated
    HAM (4/8 <-> 8/8 clock) through all of phase 2 because PE duty was
    ~60%, and it serialized startup-DMA / qkv / attention / projection.
  - engine placement: exp is ScalarE-only (the true phase-2 roofline,
    ~65us); masks/evacuations/normalize on VectorE; input x + all output
    DMAs on the GpSimd queue; weights + rec lane-shifts on the Sync queue.

Per-core PE streaming floor ~100us (qkv 41 + S^T 15 (64-row-tiled pairs)
+ ctx 29 + proj 14); ScalarE exp ~65us hides under it when interleaved.
"""

import sys

if "/opt/trn_rl_repo" not in sys.path:
    sys.path.insert(0, "/opt/trn_rl_repo")

from collections import deque
from contextlib import ExitStack

import numpy as np
import ml_dtypes

import concourse.bass as bass
import concourse.bacc as bacc
import concourse.tile as tile
from concourse import mybir
from concourse.bass_utils import run_bass_kernel_spmd

B, T, D, H, DH = 2, 2048, 1024, 16, 64
NCORES = 8
HL = 4          # heads per core
P = 128         # SBUF/PSUM partitions
QT = 512        # q tile (PSUM bank, moving free dim)
KT = 128        # kv tile (PSUM partition dim)
F32 = mybir.dt.float32
BF16 = mybir.dt.bfloat16
BF16NP = ml_dtypes.bfloat16

# emission-time engine-cost model (ns) for the deficit scheduler
def _act_cost(w):        # one 2-head exp over w columns (measured ovh ~205cyc)
    return (2 * w + 205) / 1.2

def _pe_tile_cost(w):    # S^T pair (row-tiled, concurrent) + 2 ctx matmuls
    return (w + 2 * w) / 2.4 + 80


def _build_nc() -> bass.Bass:
    nc = bacc.Bacc(None)
    Exp = mybir.ActivationFunctionType.Exp

    # host-pre-tiled inputs (partition-major, large contiguous lines)
    xt_d = nc.dram_tensor("xt", [P, 4, 8, QT], BF16, kind="ExternalInput")
    wqk_d = nc.dram_tensor("wqk", [P, 8, 512], BF16, kind="ExternalInput")
    wv_d = nc.dram_tensor("wv", [P, 8, 256], BF16, kind="ExternalInput")
    wp_d = nc.dram_tensor("wp", [P, 2, D], BF16, kind="ExternalInput")
    # bqk padded to 256B rows: 16B-row descriptors stalled the DMA queue 8us
    bqk_d = nc.dram_tensor("bqk", [P, 64], F32, kind="ExternalInput")
    bv_d = nc.dram_tensor("bv", [256], F32, kind="ExternalInput")
    out_d = nc.dram_tensor("out", [T, D], BF16, kind="ExternalOutput")

    kv = np.arange(KT)
    # 0/1 keep-mask (bf16): applied multiplicatively to P AFTER the exp, so
    # the ScalarE exp never waits on a VectorE mask add.
    tril01_np = (kv[:, None] <= kv[None, :]).astype(BF16NP)
    tril_d = nc.inline_tensor(tril01_np, name="tril01")

    with tile.TileContext(nc) as tc, ExitStack() as ctx:
        perm = ctx.enter_context(tc.tile_pool(name="perm", bufs=1))
        ppool = ctx.enter_context(tc.tile_pool(name="ppool", bufs=6))
        rpool = ctx.enter_context(tc.tile_pool(name="rpool", bufs=3))
        opool = ctx.enter_context(tc.tile_pool(name="opool", bufs=3))
        # separate PSUM rings: S tiles (4KB x2) must not share slots with
        # qkv/proj accumulators (2KB x2) — a shared ring makes every filler
        # allocation WAR-wait on an exp two slots back, collapsing the
        # attention pipeline depth. psC: ctx accumulators (2KB x2). 16KB total.
        psS = ctx.enter_context(tc.tile_pool(name="psS", bufs=2, space="PSUM"))
        psA = ctx.enter_context(tc.tile_pool(name="psA", bufs=2, space="PSUM"))
        psC = ctx.enter_context(tc.tile_pool(name="psC", bufs=2, space="PSUM"))

        # --- startup DMAs (flattened to one free dim for 4-8KB descriptors;
        # MM0 needs wqk half A + xq0: each split to its own queue so the
        # first matmul starts ~13us in (7us of that is framework preamble))
        wqk_sb = perm.tile([P, 8, 512], BF16)
        nc.sync.dma_start(
            out=wqk_sb[:, 0:4].rearrange("p a j -> p (a j)"),
            in_=wqk_d[:, 0:4].rearrange("p a j -> p (a j)"),
        )
        xq_sb = []
        for tt4 in range(4):
            xq = perm.tile([P, 8, QT], BF16, name=f"xq{tt4}")
            xq_sb.append(xq)
        nc.scalar.dma_start(
            out=xq_sb[0][:, 0:4].rearrange("p a t -> p (a t)"),
            in_=xt_d[:, 0, 0:4].rearrange("p a t -> p (a t)"),
        )
        nc.gpsimd.dma_start(
            out=xq_sb[0][:, 4:8].rearrange("p a t -> p (a t)"),
            in_=xt_d[:, 0, 4:8].rearrange("p a t -> p (a t)"),
        )
        nc.scalar.dma_start(
            out=wqk_sb[:, 4:8].rearrange("p a j -> p (a j)"),
            in_=wqk_d[:, 4:8].rearrange("p a j -> p (a j)"),
        )
        for tt4 in range(1, 4):
            nc.gpsimd.dma_start(
                out=xq_sb[tt4].rearrange("p a t -> p (a t)"),
                in_=xt_d[:, tt4].rearrange("p a t -> p (a t)"),
            )
        bqk_sb = perm.tile([P, 64], F32)
        nc.sync.dma_start(out=bqk_sb, in_=bqk_d[...])
        wv_sb = perm.tile([P, 8, 256], BF16)
        nc.sync.dma_start(
            out=wv_sb.rearrange("p a j -> p (a j)"),
            in_=wv_d[...].rearrange("p a j -> p (a j)"),
        )
        bv_sb = perm.tile([P, 256], F32)
        nc.sync.dma_start(
            out=bv_sb, in_=bass.AP(tensor=bv_d, offset=0, ap=[[0, P], [1, 256]])
        )
        tril_sb = perm.tile([P, KT], BF16)
        nc.sync.dma_start(out=tril_sb, in_=tril_d[...])
        wp_sb = perm.tile([P, 2, D], BF16)
        nc.sync.dma_start(
            out=wp_sb.rearrange("p a e -> p (a e)"),
            in_=wp_d[...].rearrange("p a e -> p (a e)"),
        )

        # first Exp triggers a ~2.7us ACT table load; fire it early
        expwarm = perm.tile([P, 8], F32)
        nc.scalar.activation(out=expwarm, in_=tril_sb[:, 0:8], func=Exp)

        qk_sb = perm.tile([P, 4, T], BF16)           # jt 0,1: Q^T; 2,3: K^T
        vaug_sb = perm.tile([P, 16, HL, 128], BF16)  # [kv, tt, h, V|ones]
        vaug_hh = vaug_sb.rearrange("p t (hp hh) c -> p t hp hh c", hh=2)
        ctxn_sb = perm.tile([P, 2, T], BF16)         # normalized ctx^T

        # ones halves of vaug never change: write them once.
        # even heads are [V|ones], odd heads [ones|V] (parity keeps ctx rows
        # lane-aligned with the ctxn head-pair packing; see normalize()).
        for par, sl in ((0, slice(64, 128)), (1, slice(0, 64))):
            src = bass.AP(
                tensor=tril_sb.tensor, offset=tril_sb.offset,
                ap=[tril_sb.ap[0], [0, 16], [0, 2], [1, 64]],
            )
            nc.vector.tensor_scalar(
                out=vaug_hh[:, :, :, par, sl], in0=src,
                scalar1=0.0, scalar2=1.0,
                op0=mybir.AluOpType.mult, op1=mybir.AluOpType.add,
            )

        # --- work units -------------------------------------------------
        def qk_jt(tt4, jt):
            """qkv projection, one 128-wide j chunk x 512 t: 8 MMs + evac."""
            ps = psA.tile([P, QT], F32, name=f"qkps{tt4}_{jt}", tag="acc")
            for dt in range(8):
                nc.tensor.matmul(
                    out=ps,
                    lhsT=wqk_sb[:, dt, jt * 128:(jt + 1) * 128],
                    rhs=xq_sb[tt4][:, dt, :],
                    start=(dt == 0),
                    stop=(dt == 7),
                )
            # PSUM->SBUF evac + bias on DVE: keeping this OFF ScalarE matters
            # (an Ident here head-of-line blocks ready exps in the ACT FIFO
            # behind this tile's still-accumulating matmuls)
            b = bqk_sb[:, jt:jt + 1]
            bias = bass.AP(tensor=b.tensor, offset=b.offset, ap=[b.ap[0], [0, QT]])
            nc.vector.tensor_add(
                out=qk_sb[:, jt, tt4 * QT:(tt4 + 1) * QT], in0=ps, in1=bias
            )

        def v_k(tt4, k):
            """v projection for one 128-t block: 8 MMs + vaug fills."""
            tt = tt4 * 4 + k
            psv = psA.tile([P, 256], F32, name=f"vps{tt}", tag="acc")
            for dt in range(8):
                nc.tensor.matmul(
                    out=psv,
                    lhsT=xq_sb[tt4][:, dt, k * 128:(k + 1) * 128],
                    rhs=wv_sb[:, dt, :],
                    start=(dt == 0),
                    stop=(dt == 7),
                )
            vview = vaug_hh[:, tt]
            pview = psv.rearrange("p (hp hh d) -> p hp hh d", hp=2, hh=2)
            bview = bv_sb.rearrange("p (hp hh d) -> p hp hh d", hp=2, hh=2)
            nc.vector.tensor_add(
                out=vview[:, :, 0, 0:64], in0=pview[:, :, 0, :], in1=bview[:, :, 0, :]
            )
            nc.vector.tensor_add(
                out=vview[:, :, 1, 64:128], in0=pview[:, :, 1, :], in1=bview[:, :, 1, :]
            )

        def proj_block(tt, fine=False):
            """output projection for one 128-t block + DMA out. fine=True
            (tail blocks) pipelines per-half evac+DMA to shorten the drain."""
            ob = opool.tile([P, D], BF16, name=f"ob{tt}", tag="ob")
            for et in range(2):
                ps = psA.tile([P, QT], F32, name=f"ops{tt}_{et}", tag="acc")
                for ft in range(2):
                    nc.tensor.matmul(
                        out=ps,
                        lhsT=ctxn_sb[:, ft, tt * KT:(tt + 1) * KT],
                        rhs=wp_sb[:, ft, et * QT:(et + 1) * QT],
                        start=(ft == 0),
                        stop=(ft == 1),
                    )
                if fine:
                    # tail blocks: evac via the (now exp-free) ScalarE and
                    # split the out-DMA per half to shorten the drain
                    nc.scalar.activation(
                        out=ob[:, et * QT:(et + 1) * QT], in_=ps,
                        func=mybir.ActivationFunctionType.Identity, scale=1.0,
                    )
                    q = nc.gpsimd if et == 0 else nc.scalar
                    q.dma_start(
                        out=out_d[tt * KT:(tt + 1) * KT, et * QT:(et + 1) * QT],
                        in_=ob[:, et * QT:(et + 1) * QT],
                    )
                else:
                    nc.vector.tensor_copy(out=ob[:, et * QT:(et + 1) * QT], in_=ps)
            if not fine:
                nc.gpsimd.dma_start(out=out_d[tt * KT:(tt + 1) * KT, :], in_=ob)

        # --- attention stages -------------------------------------------
        Th_by_blk = {}

        def stage_a_mm(hp, qi, j):
            """S^T pair: 64-row-tiled, the two heads run concurrently."""
            q0 = qi * QT
            qoff = max(0, KT * j - q0)
            s = psS.tile([P, 2, QT], F32, name=f"s{hp}_{qi}_{j}", tag="s")
            for hh in range(2):
                nc.tensor.matmul(
                    out=s[:, hh, qoff:QT],
                    lhsT=qk_sb[hh * 64:(hh + 1) * 64, 2 + hp, j * KT:(j + 1) * KT],
                    rhs=qk_sb[hh * 64:(hh + 1) * 64, hp, q0 + qoff:q0 + QT],
                    start=True,
                    stop=True,
                )
            return s

        def stage_a_post(hp, qi, j, s):
            """2-head exp (+ diagonal 0/1 mask-mul AFTER the exp, so ACT
            never waits on the DVE mask)."""
            q0 = qi * QT
            qoff = max(0, KT * j - q0)
            p_t = ppool.tile([P, 2, QT], BF16, name=f"p{hp}_{qi}_{j}", tag="p")
            nc.scalar.activation(
                out=p_t[:, :, qoff:QT], in_=s[:, :, qoff:QT], func=Exp
            )
            if j >= 4 * qi:
                pm = p_t[:, :, qoff:qoff + KT]
                mask_b = bass.AP(
                    tensor=tril_sb.tensor, offset=tril_sb.offset,
                    ap=[tril_sb.ap[0], [0, 2], [1, KT]],
                )
                nc.vector.tensor_mul(out=pm, in0=pm, in1=mask_b)
            return p_t

        def stage_b(hp, qi, j, p_t):
            """[ctx|den] matmuls for both heads; normalize at last j."""
            q0 = qi * QT
            njt = 4 * qi + 4
            qoff = max(0, KT * j - q0)
            if j == 0:
                Th_by_blk[(hp, qi)] = [
                    psC.tile([P, QT], F32, name=f"T{hp}_{qi}_{hh}", tag="C")
                    for hh in range(2)
                ]
            Th = Th_by_blk[(hp, qi)]
            for hh in range(2):
                nc.tensor.matmul(
                    out=Th[hh][:, qoff:QT],
                    lhsT=vaug_sb[:, j, hp * 2 + hh, :],
                    rhs=p_t[:, hh, qoff:QT],
                    start=(j == 0),
                    stop=(j == njt - 1),
                )
            if j == njt - 1:
                normalize(hp, qi)

        def normalize(hp, qi):
            q0 = qi * QT
            Th = Th_by_blk.pop((hp, qi))
            for hh in range(2):
                cl = hh * 64          # ctx lanes base
                rec = rpool.tile([P, QT], F32, name=f"rec{hp}_{qi}_{hh}", tag="rec")
                # reciprocal_approx_fast mis-executes at partition base 64
                # (HW-verified), so always run it at base 0.
                if hh == 1:
                    nc.vector.reciprocal_approx_fast(out=rec[0:64, :], in_=Th[hh][0:64, :])
                    nc.sync.dma_start(out=rec[64:128, :], in_=rec[0:64, :])
                else:
                    nc.vector.tensor_copy(out=rec[64:128, :], in_=Th[hh][64:128, :])
                    nc.sync.dma_start(out=rec[0:64, :], in_=rec[64:128, :])
                    nc.vector.reciprocal_approx_fast(out=rec[0:64, :], in_=rec[0:64, :])
                nc.vector.tensor_mul(
                    out=ctxn_sb[cl:cl + 64, hp, q0:q0 + QT],
                    in0=Th[hh][cl:cl + 64, :],
                    in1=rec[cl:cl + 64, :],
                )

        # --- globally interleaved emission ------------------------------
        # fillers: (tag, pe_cost_ns, emit_fn) of ready PE work injected
        # between attention j-tiles to cover the ACT(exp) deficit. proj
        # blocks are held in a reserve and only spent inside attn(3) (the
        # qkv quarters cover attn(0..2)); attn(3) has no quarter left.
        fillers = deque()
        reserve = deque()

        def add_quarter_fillers(tt4):
            for jt in range(4):
                fillers.append((("q", tt4), 1750.0, lambda j=jt: qk_jt(tt4, j)))
            for k in range(4):
                fillers.append((("q", tt4), 950.0, lambda kk=k: v_k(tt4, kk)))

        def add_proj_reserve(qi):
            for tt in range(qi * 4, qi * 4 + 4):
                reserve.append(
                    (("pj", qi), 950.0, lambda t=tt: proj_block(t, fine=(qi == 3)))
                )

        def drain_tag(tag):
            while fillers and any(f[0] == tag for f in fillers):
                t, c, fn = fillers.popleft()
                fn()

        def attn_block(qi, use_reserve):
            # j-tiles processed in PAIRS: the 4 S^T matmuls of two tiles are
            # emitted adjacently so only one 64-row-mode entry/exit (~100ns
            # each) is paid per pair instead of per tile; the 2-slot S ring
            # holds exactly one pair. Fillers go between the exps and the
            # ctx matmuls so pending exps get PE-time cover.
            lag = 0.0
            for hp in range(2):
                pend = deque()
                for k in range((4 * qi + 4) // 2):
                    j0, j1 = 2 * k, 2 * k + 1
                    s0 = stage_a_mm(hp, qi, j0)
                    s1 = stage_a_mm(hp, qi, j1)
                    p0 = stage_a_post(hp, qi, j0, s0)
                    p1 = stage_a_post(hp, qi, j1, s1)
                    pend.append((hp, qi, j0, p0))
                    pend.append((hp, qi, j1, p1))
                    for j in (j0, j1):
                        w = QT - max(0, KT * j - qi * QT)
                        lag += _act_cost(w) - _pe_tile_cost(w)
                    while lag > 400.0 and (fillers or (use_reserve and reserve)):
                        t, c, fn = (fillers or reserve).popleft()
                        fn()
                        lag -= c
                    while len(pend) > 2:
                        stage_b(*pend.popleft())
                while pend:
                    stage_b(*pend.popleft())

        # quarter 0 dense, then attention blocks with filler injection;
        # proj(0..2) are reserved for exp-heavy attn(3), proj(3) is the tail
        add_quarter_fillers(0)
        drain_tag(("q", 0))
        for qi in range(4):
            if qi < 3:
                add_quarter_fillers(qi + 1)
            if qi >= 1:
                add_proj_reserve(qi - 1)   # ready once attn(qi-1) normalized
            attn_block(qi, use_reserve=(qi == 3))
            if qi < 3:
                drain_tag(("q", qi + 1))   # quarter qi+1 must precede attn qi+1
        add_proj_reserve(3)
        for t, c, fn in list(fillers) + list(reserve):
            fn()

    nc.finalize()
    return nc


_NC_CACHE: list = []


def _get_nc() -> bass.Bass:
    if not _NC_CACHE:
        _NC_CACHE.append(_build_nc())
    return _NC_CACHE[0]


def _shard_inputs(x, W_qkv, b_qkv, W_proj):
    scale = np.float32(1.0 / np.sqrt(DH))
    in_maps = []
    # x pre-tiled per batch: [p, quarter, dt, t] with 8KB contiguous lines
    xts = []
    for b in range(B):
        xt = x[b].T.reshape(8, P, 4, QT).transpose(1, 2, 0, 3)
        xts.append(np.ascontiguousarray(xt).astype(BF16NP))
    for c in range(NCORES):
        b = c // 4
        h0 = (c % 4) * HL
        lo = h0 * DH
        wqk = np.concatenate(
            [W_qkv[:, lo:lo + 256] * scale, W_qkv[:, D + lo:D + lo + 256]], axis=1
        )
        bqk = np.concatenate([b_qkv[lo:lo + 256] * scale, b_qkv[D + lo:D + lo + 256]])
        wv = W_qkv[:, 2 * D + lo:2 * D + lo + 256]
        wp = W_proj[lo:lo + 256, :]
        in_maps.append({
            "xt": xts[b],
            "wqk": np.ascontiguousarray(
                wqk.reshape(8, P, 512).transpose(1, 0, 2)).astype(BF16NP),
            "wv": np.ascontiguousarray(
                wv.reshape(8, P, 256).transpose(1, 0, 2)).astype(BF16NP),
            "wp": np.ascontiguousarray(
                wp.reshape(2, P, D).transpose(1, 0, 2)).astype(BF16NP),
            "bqk": np.ascontiguousarray(
                np.pad(bqk.reshape(4, P).T, ((0, 0), (0, 60))), np.float32),
            "bv": np.ascontiguousarray(b_qkv[2 * D + lo:2 * D + lo + 256], np.float32),
        })
    return in_maps


def kernel(x, W_qkv, b_qkv, W_proj, b_proj, _trace=False, _tmpdir=None):
    x = np.asarray(x, np.float32)
    W_qkv = np.asarray(W_qkv, np.float32)
    b_qkv = np.asarray(b_qkv, np.float32)
    W_proj = np.asarray(W_proj, np.float32)
    b_proj = np.asarray(b_proj, np.float32)

    nc = _get_nc()
    in_maps = _shard_inputs(x, W_qkv, b_qkv, W_proj)
    kw = {}
    if _trace:
        kw = dict(trace=True, tmpdir=_tmpdir)
    r = run_bass_kernel_spmd(nc, in_maps, core_ids=list(range(NCORES)), **kw)
    out = np.zeros((B, T, D), np.float32)
    for c in range(NCORES):
        out[c // 4] += np.asarray(r.results[c]["out"], np.float32)
    out += b_proj[None, None, :]
    if _trace:
        return out, r
    return out
